# revision 18
# baseline (speedup 1.0000x reference)
"""AdaptiveFusion kernel for 8 TRN2 NeuronCores.

Computes, for xs [V=3, N=131072, D=512], alpha_w [512], alpha_b [1]:
    logits = leaky_relu(einsum('vnd,d->vn', xs, alpha_w) + alpha_b, 0.01)
    attn   = softmax(logits, axis=0)           # over the V=3 views
    out    = einsum('vn,vnd->nd', attn, xs)    # [N, D]

Data-parallel over the node axis N: each of the 8 cores handles
N_local = 16384 nodes; alpha_w/alpha_b replicated; no collectives.

Per-core pipeline (tiles of 128 nodes, natural [n, d] layout):
  - one DMA loads all 3 views' rows for the tile  [128, 3*512] f32
  - VectorE tensor_tensor_reduce (x * w, +reduce) -> per-node dot
  - ScalarE Lrelu(dot + b), then Exp with accum_out -> denominator
  - VectorE builds diag(e_v) = identity * e_v  (tensor_scalar_mul)
  - TensorE: psum += diag(e_v) @ x_v  (float32r, full rate) -- does the
    per-row scale AND the view sum in PSUM
  - ScalarE Copy psum->sbuf with scale = 1/denom (fused normalize)
  - DMA out
"""

import numpy as np
from contextlib import ExitStack

import concourse.bass as bass
import concourse.tile as tile
from concourse import bacc, mybir
from concourse.bass_utils import run_bass_kernel_spmd

V = 3
N = 131072
D = 512
NCORES = 8
NL = N // NCORES          # 16384 nodes per core
TILE_N = 128              # nodes per tile (partition dim)
NT = NL // TILE_N         # 128 tiles per core
NEG_SLOPE = 0.01

F32 = mybir.dt.float32
BF16 = mybir.dt.bfloat16
AF = mybir.ActivationFunctionType
ALU = mybir.AluOpType


def build_nc(reps: int = 1) -> bass.Bass:
    nc = bacc.Bacc("TRN2", target_bir_lowering=False, debug=False)
    xs = nc.declare_dram_parameter("xs", [V, NL, D], F32, isOutput=False)
    aw = nc.declare_dram_parameter("alpha_w", [1, D], F32, isOutput=False)
    ab = nc.declare_dram_parameter("alpha_b", [1, 1], F32, isOutput=False)
    out = nc.declare_dram_parameter("out", [NL, D], BF16, isOutput=True)

    GRP = 4  # n-tiles per group (batches the small softmax ops)

    with ExitStack() as ctx:
        tc = ctx.enter_context(tile.TileContext(nc))
        const_pool = ctx.enter_context(tc.tile_pool(name="const", bufs=1))
        x_pool = ctx.enter_context(tc.tile_pool(name="x", bufs=10))
        scr_pool = ctx.enter_context(tc.tile_pool(name="scr", bufs=8))
        sm_pool = ctx.enter_context(tc.tile_pool(name="sm", bufs=4))
        diag_pool = ctx.enter_context(tc.tile_pool(name="diag", bufs=8))
        out_pool = ctx.enter_context(tc.tile_pool(name="outp", bufs=8))
        psum_pool = ctx.enter_context(tc.tile_pool(name="psum", bufs=4, space="PSUM"))

        # ---- constants ----
        w_stage = const_pool.tile([128, D], BF16)
        nc.gpsimd.dma_start(w_stage[0:1, :], aw[:, :])  # f32 -> bf16 cast
        w_bc = const_pool.tile([128, D], BF16)
        nc.gpsimd.partition_broadcast(w_bc[:, :], w_stage[0:1, :])

        b_stage = const_pool.tile([128, 1], F32)
        nc.sync.dma_start(b_stage[0:1, :], ab[:, :])
        b_col = const_pool.tile([128, 1], F32)
        nc.gpsimd.partition_broadcast(b_col[:, :], b_stage[0:1, :])

        ones = const_pool.tile([128, 128], BF16)
        nc.vector.memset(ones[:, :], 1.0)
        ident = const_pool.tile([128, 128], BF16)
        # iota[p, f] = f - p ; keep where == 0 -> identity matrix
        nc.gpsimd.affine_select(
            ident[:, :], ones[:, :], pattern=[[1, 128]],
            compare_op=ALU.is_equal, fill=0.0, base=0, channel_multiplier=-1,
        )

        # ---- main loop over groups of GRP node tiles ----
        for g in range((NT // GRP) * reps):
            g = g % (NT // GRP)
            lgt = sm_pool.tile([128, V * GRP], F32)
            x_ts = []
            for j in range(GRP):
                n0 = (g * GRP + j) * TILE_N
                x_t = x_pool.tile([128, V * D], BF16)
                x_ts.append(x_t)
                src = xs[:, n0:n0 + TILE_N, :].rearrange("v n d -> n v d")
                dst = x_t[:, :].rearrange("p (v d) -> p v d", v=V)
                nc.gpsimd.dma_start(dst, src)  # f32 -> bf16 cast during DMA

                for v in range(V):
                    scr = scr_pool.tile([128, D], BF16)
                    # out = (x * 1.0) * w ; accum_out = sum(out) = <x, w>
                    nc.vector.scalar_tensor_tensor(
                        out=scr[:, :],
                        in0=x_t[:, v * D:(v + 1) * D],
                        scalar=1.0,
                        in1=w_bc[:, :],
                        op0=ALU.mult,
                        op1=ALU.mult,
                        accum_out=lgt[:, j * V + v:j * V + v + 1],
                    )

            # batched leaky_relu(dot + b): z = dot+b ; lrl = max(z, 0.01*z)
            z_t = sm_pool.tile([128, V * GRP], F32)
            nc.vector.tensor_scalar_add(z_t[:, :], lgt[:, :], b_col[:, :])
            z1_t = sm_pool.tile([128, V * GRP], F32)
            nc.vector.tensor_scalar_mul(z1_t[:, :], z_t[:, :], NEG_SLOPE)
            lrl = sm_pool.tile([128, V * GRP], F32)
            nc.vector.tensor_max(lrl[:, :], z_t[:, :], z1_t[:, :])

            e_t = sm_pool.tile([128, V * GRP], F32)
            nc.scalar.activation(e_t[:, :], lrl[:, :], AF.Exp)
            den = sm_pool.tile([128, GRP], F32)
            nc.vector.tensor_reduce(
                den[:, :], e_t[:, :].rearrange("p (j v) -> p j v", v=V),
                axis=mybir.AxisListType.X, op=ALU.add,
            )
            rc = sm_pool.tile([128, GRP], F32)
            nc.vector.reciprocal(rc[:, :], den[:, :])

            for j in range(GRP):
                n0 = (g * GRP + j) * TILE_N
                x_t = x_ts[j]
                diag = diag_pool.tile([128, V * 128], BF16)
                # build diag(e_v) on three different engines to balance load
                nc.vector.tensor_scalar_mul(
                    diag[:, 0:128], ident[:, :], e_t[:, j * V:j * V + 1],
                )
                nc.scalar.activation(
                    diag[:, 128:256], ident[:, :], AF.Copy, bias=0.0,
                    scale=e_t[:, j * V + 1:j * V + 2],
                )
                nc.gpsimd.tensor_scalar_mul(
                    diag[:, 256:384], ident[:, :], e_t[:, j * V + 2:j * V + 3],
                )

                ps = psum_pool.tile([128, D], F32)
                for v in range(V):
                    nc.tensor.matmul(
                        ps[:, :],
                        diag[:, v * 128:(v + 1) * 128],
                        x_t[:, v * D:(v + 1) * D],
                        start=(v == 0),
                        stop=(v == V - 1),
                    )

                o_t = out_pool.tile([128, D], BF16)
                nc.scalar.activation(
                    o_t[:, :], ps[:, :], AF.Copy, bias=0.0, scale=rc[:, j:j + 1],
                )
                nc.sync.dma_start(out[n0:n0 + TILE_N, :], o_t[:, :])

    nc.compile()
    return nc


def _make_in_maps(xs, alpha_w, alpha_b):
    xs = np.asarray(xs, dtype=np.float32)
    aw = np.asarray(alpha_w, dtype=np.float32).reshape(1, D)
    ab = np.asarray(alpha_b, dtype=np.float32).reshape(1, 1)
    in_maps = []
    for i in range(NCORES):
        in_maps.append({
            "xs": np.ascontiguousarray(xs[:, i * NL:(i + 1) * NL, :]),
            "alpha_w": aw,
            "alpha_b": ab,
        })
    return in_maps


def run(xs, alpha_w, alpha_b, trace=False):
    nc = build_nc()
    in_maps = _make_in_maps(xs, alpha_w, alpha_b)
    res = run_bass_kernel_spmd(nc, in_maps, list(range(NCORES)), trace=trace)
    out = np.concatenate(
        [np.asarray(res.results[i]["out"]) for i in range(NCORES)], axis=0
    ).astype(np.float32)
    return out, res


def kernel(xs, alpha_w, alpha_b):
    out, _ = run(xs, alpha_w, alpha_b, trace=False)
    return out


# revision 19
# speedup vs baseline: 1.5849x; 1.5849x over previous
"""AdaptiveFusion kernel for 8 TRN2 NeuronCores.

Computes, for xs [V=3, N=131072, D=512], alpha_w [512], alpha_b [1]:
    logits = leaky_relu(einsum('vnd,d->vn', xs, alpha_w) + alpha_b, 0.01)
    attn   = softmax(logits, axis=0)           # over the V=3 views
    out    = einsum('vn,vnd->nd', attn, xs)    # [N, D]

Data-parallel over the node axis N: each of the 8 cores handles
N_local = 16384 nodes; alpha_w/alpha_b replicated; no collectives.

Per-core pipeline (tiles of 128 nodes, natural [n, d] layout):
  - one DMA loads all 3 views' rows for the tile  [128, 3*512] f32
  - VectorE tensor_tensor_reduce (x * w, +reduce) -> per-node dot
  - ScalarE Lrelu(dot + b), then Exp with accum_out -> denominator
  - VectorE builds diag(e_v) = identity * e_v  (tensor_scalar_mul)
  - TensorE: psum += diag(e_v) @ x_v  (float32r, full rate) -- does the
    per-row scale AND the view sum in PSUM
  - ScalarE Copy psum->sbuf with scale = 1/denom (fused normalize)
  - DMA out
"""

import numpy as np
from contextlib import ExitStack

import concourse.bass as bass
import concourse.tile as tile
from concourse import bacc, mybir
from concourse.bass_utils import run_bass_kernel_spmd

V = 3
N = 131072
D = 512
NCORES = 8
NL = N // NCORES          # 16384 nodes per core
TILE_N = 128              # nodes per tile (partition dim)
NT = NL // TILE_N         # 128 tiles per core
NEG_SLOPE = 0.01

F32 = mybir.dt.float32
BF16 = mybir.dt.bfloat16
AF = mybir.ActivationFunctionType
ALU = mybir.AluOpType


def build_nc(reps: int = 1) -> bass.Bass:
    nc = bacc.Bacc("TRN2", target_bir_lowering=False, debug=False)
    xs = nc.declare_dram_parameter("xs", [V, NL, D], F32, isOutput=False)
    aw = nc.declare_dram_parameter("alpha_w", [1, D], F32, isOutput=False)
    ab = nc.declare_dram_parameter("alpha_b", [1, 1], F32, isOutput=False)
    out = nc.declare_dram_parameter("out", [NL, D], BF16, isOutput=True)

    GRP = 4  # n-tiles per group (batches the small softmax ops)

    with ExitStack() as ctx:
        tc = ctx.enter_context(tile.TileContext(nc))
        const_pool = ctx.enter_context(tc.tile_pool(name="const", bufs=1))
        x_pool = ctx.enter_context(tc.tile_pool(name="x", bufs=10))
        scr_pool = ctx.enter_context(tc.tile_pool(name="scr", bufs=8))
        sm_pool = ctx.enter_context(tc.tile_pool(name="sm", bufs=4))
        diag_pool = ctx.enter_context(tc.tile_pool(name="diag", bufs=8))
        out_pool = ctx.enter_context(tc.tile_pool(name="outp", bufs=8))
        psum_pool = ctx.enter_context(tc.tile_pool(name="psum", bufs=4, space="PSUM"))

        # ---- constants ----
        w_stage = const_pool.tile([128, D], BF16)
        nc.gpsimd.dma_start(w_stage[0:1, :], aw[:, :])  # f32 -> bf16 cast
        w_bc = const_pool.tile([128, D], BF16)
        nc.gpsimd.partition_broadcast(w_bc[:, :], w_stage[0:1, :])

        b_stage = const_pool.tile([128, 1], F32)
        nc.sync.dma_start(b_stage[0:1, :], ab[:, :])
        b_col = const_pool.tile([128, 1], F32)
        nc.gpsimd.partition_broadcast(b_col[:, :], b_stage[0:1, :])

        ones = const_pool.tile([128, 128], BF16)
        nc.vector.memset(ones[:, :], 1.0)
        ident = const_pool.tile([128, 128], BF16)
        # iota[p, f] = f - p ; keep where == 0 -> identity matrix
        nc.gpsimd.affine_select(
            ident[:, :], ones[:, :], pattern=[[1, 128]],
            compare_op=ALU.is_equal, fill=0.0, base=0, channel_multiplier=-1,
        )

        # ---- main loop over groups of GRP node tiles ----
        for g in range((NT // GRP) * reps):
            g = g % (NT // GRP)
            lgt = sm_pool.tile([128, V * GRP], F32)
            x_ts = []
            for j in range(GRP):
                n0 = (g * GRP + j) * TILE_N
                x_t = x_pool.tile([128, V * D], BF16)
                x_ts.append(x_t)
                src = xs[:, n0:n0 + TILE_N, :].rearrange("v n d -> n v d")
                dst = x_t[:, :].rearrange("p (v d) -> p v d", v=V)
                nc.gpsimd.dma_start(dst, src)  # f32 -> bf16 cast during DMA

                for v in range(V):
                    scr = scr_pool.tile([128, D], BF16)
                    # out = (x * 1.0) * w ; accum_out = sum(out) = <x, w>
                    nc.vector.scalar_tensor_tensor(
                        out=scr[:, :],
                        in0=x_t[:, v * D:(v + 1) * D],
                        scalar=1.0,
                        in1=w_bc[:, :],
                        op0=ALU.mult,
                        op1=ALU.mult,
                        accum_out=lgt[:, j * V + v:j * V + v + 1],
                    )

            # batched leaky_relu(dot + b): z = dot+b ; lrl = max(z, 0.01*z)
            z_t = sm_pool.tile([128, V * GRP], F32)
            nc.vector.tensor_scalar_add(z_t[:, :], lgt[:, :], b_col[:, :])
            z1_t = sm_pool.tile([128, V * GRP], F32)
            nc.vector.tensor_scalar_mul(z1_t[:, :], z_t[:, :], NEG_SLOPE)
            lrl = sm_pool.tile([128, V * GRP], F32)
            nc.vector.tensor_max(lrl[:, :], z_t[:, :], z1_t[:, :])

            e_t = sm_pool.tile([128, V * GRP], F32)
            nc.scalar.activation(e_t[:, :], lrl[:, :], AF.Exp)
            den = sm_pool.tile([128, GRP], F32)
            nc.vector.tensor_reduce(
                den[:, :], e_t[:, :].rearrange("p (j v) -> p j v", v=V),
                axis=mybir.AxisListType.X, op=ALU.add,
            )
            rc = sm_pool.tile([128, GRP], F32)
            nc.vector.reciprocal(rc[:, :], den[:, :])

            for j in range(GRP):
                n0 = (g * GRP + j) * TILE_N
                x_t = x_ts[j]
                diag = diag_pool.tile([128, V * 128], BF16)
                # build diag(e_v) on three different engines to balance load
                nc.vector.tensor_scalar_mul(
                    diag[:, 0:128], ident[:, :], e_t[:, j * V:j * V + 1],
                )
                nc.scalar.activation(
                    diag[:, 128:256], ident[:, :], AF.Copy, bias=0.0,
                    scale=e_t[:, j * V + 1:j * V + 2],
                )
                nc.vector.tensor_scalar_mul(
                    diag[:, 256:384], ident[:, :], e_t[:, j * V + 2:j * V + 3],
                )

                ps = psum_pool.tile([128, D], F32)
                for v in range(V):
                    nc.tensor.matmul(
                        ps[:, :],
                        diag[:, v * 128:(v + 1) * 128],
                        x_t[:, v * D:(v + 1) * D],
                        start=(v == 0),
                        stop=(v == V - 1),
                    )

                o_t = out_pool.tile([128, D], BF16)
                nc.scalar.activation(
                    o_t[:, :], ps[:, :], AF.Copy, bias=0.0, scale=rc[:, j:j + 1],
                )
                nc.sync.dma_start(out[n0:n0 + TILE_N, :], o_t[:, :])

    nc.compile()
    return nc


def _make_in_maps(xs, alpha_w, alpha_b):
    xs = np.asarray(xs, dtype=np.float32)
    aw = np.asarray(alpha_w, dtype=np.float32).reshape(1, D)
    ab = np.asarray(alpha_b, dtype=np.float32).reshape(1, 1)
    in_maps = []
    for i in range(NCORES):
        in_maps.append({
            "xs": np.ascontiguousarray(xs[:, i * NL:(i + 1) * NL, :]),
            "alpha_w": aw,
            "alpha_b": ab,
        })
    return in_maps


def run(xs, alpha_w, alpha_b, trace=False):
    nc = build_nc()
    in_maps = _make_in_maps(xs, alpha_w, alpha_b)
    res = run_bass_kernel_spmd(nc, in_maps, list(range(NCORES)), trace=trace)
    out = np.concatenate(
        [np.asarray(res.results[i]["out"]) for i in range(NCORES)], axis=0
    ).astype(np.float32)
    return out, res


def kernel(xs, alpha_w, alpha_b):
    out, _ = run(xs, alpha_w, alpha_b, trace=False)
    return out
